# revision 15
# baseline (speedup 1.0000x reference)
"""PairwiseConv1D (valid 1D conv, NWC x WIO -> NWC) on 8 TRN2 NeuronCores.

Strategy (v4: bf16 inputs, full-PSUM k-inner sweeps, 3-engine drains):
  - Data-parallel over batch: B=32 -> 4 batches per core, kernel replicated.
  - Host casts x and w to bf16 and feeds x transposed per batch ([C, L]) so
    the contraction dim C sits on SBUF partitions. Input HBM traffic halves
    (2.3e-3 rel err); output stays f32 (dtype-converting PSUM drains
    measured slower and stall the PE).
  - out.T[f, i] = sum_k w[k].T @ xT[:, i+k] as 7 accumulating matmuls per
    512-wide output chunk (PSUM bank = 512 fp32).
  - k-inner sweeps over all 8 PSUM banks (for k: for bank j: matmul): each
    stationary w[k] serves 8*512 moving columns per load, halving the
    serial weight-reload cost vs a 4-chunk interleave (loads measured
    non-overlappable: explicit ldweights prefetch gained nothing).
  - The 8-bank end-of-sweep drain burst is met by splitting the f32
    PSUM->SBUF drains across three engines: DVE {0,3,6}, ACT {1,4,7},
    GPSIMD {2,5}; each bank's drain must land within ~1.5us of the bank
    completing, before the next sweep reuses the bank.
  - All HBM DMA (x loads, weight load, output stores) issued from SP.
"""

import numpy as np
import ml_dtypes

import concourse.bass as bass
import concourse.mybir as mybir
from concourse.bass_utils import run_bass_kernel_spmd

B, L, C, K, F = 32, 8192, 128, 7, 128
NCORES = 8
BPC = B // NCORES  # batches per core
LOUT = L - K + 1  # 8186
CHUNK = 512  # PSUM bank width in fp32
NBANK = 8  # PSUM banks = chunks per sweep
NCHUNK = (LOUT + CHUNK - 1) // CHUNK  # 16 chunks per batch, last = 506
NSWEEP = NCHUNK // NBANK  # 2 sweeps per batch
SWCOLS = NBANK * CHUNK  # 4096 output cols per sweep / store group
XDMA = 4  # x-load DMAs per batch
XCOLS = L // XDMA  # 2048 cols per x DMA
NOBUF = 2  # output group slots (1 group = 1 sweep)

# drain engine assignment: bank j -> (engine, index within sweep)
# (GPSIMD cannot access PSUM, so only DVE and ACT can drain)
DVE_BANKS = (0, 2, 4, 6)
ACT_BANKS = (1, 3, 5, 7)

BF16 = mybir.dt.bfloat16
NPBF16 = ml_dtypes.bfloat16

_nc = None


def _chunk_n(cj):
    return CHUNK if cj < NCHUNK - 1 else LOUT - (NCHUNK - 1) * CHUNK


def _build(reps=1, detect_races=True):
    f32 = mybir.dt.float32
    nc = bass.Bass(detect_race_conditions=detect_races)
    xT = nc.dram_tensor("xT", [BPC, C, L], BF16, kind="ExternalInput")
    w = nc.dram_tensor("w", [K, C, F], BF16, kind="ExternalInput")
    outT = nc.dram_tensor("outT", [BPC, F, LOUT], f32, kind="ExternalOutput")

    G = reps * BPC  # total batch passes
    TTS = G * NSWEEP  # total sweeps

    from contextlib import ExitStack

    with ExitStack() as ctx:
        wsb = ctx.enter_context(nc.sbuf_tensor([C, K * F], BF16))
        xbuf0 = ctx.enter_context(nc.sbuf_tensor([C, L], BF16))
        xbuf1 = ctx.enter_context(nc.sbuf_tensor([C, L], BF16))
        obuf = ctx.enter_context(nc.sbuf_tensor([F, NOBUF * SWCOLS], f32))
        psum = ctx.enter_context(nc.psum_tensor([F, NBANK * CHUNK], f32))
        wsem = ctx.enter_context(nc.semaphore())
        # per-x-DMA-slot sems: counting one sem per slot keeps waits safe
        # against out-of-order completion across DMA queues
        xsems = [
            ctx.enter_context(nc.semaphore(name=f"xsem{c}")) for c in range(XDMA)
        ]
        pe_sem = ctx.enter_context(nc.semaphore())
        dve_sem = ctx.enter_context(nc.semaphore())
        act_sem = ctx.enter_context(nc.semaphore())
        osems = [
            ctx.enter_context(nc.semaphore(name=f"osem{s}")) for s in range(NOBUF)
        ]
        block = ctx.enter_context(nc.Block())

        xbufs = [xbuf0, xbuf1]

        def drain_wait(engine, S, j):
            # bank j of sweep S drained?
            for sem, banks in (
                (dve_sem, DVE_BANKS),
                (act_sem, ACT_BANKS),
            ):
                if j in banks:
                    engine.wait_ge(sem, len(banks) * S + banks.index(j) + 1)
                    return

        def _store(engine, S):
            b = (S // NSWEEP) % BPC
            s = S % NSWEEP
            cols0 = s * SWCOLS
            ncols = min(SWCOLS, LOUT - cols0)
            slot = S % NOBUF
            engine.wait_ge(dve_sem, len(DVE_BANKS) * (S + 1))
            engine.dma_start(
                outT[b, :, cols0 : cols0 + ncols],
                obuf[:, slot * SWCOLS : slot * SWCOLS + ncols],
            ).then_inc(osems[slot], 16)

        @block.sync
        def _(sync):
            # weights: [K, C, F] -> SBUF [C, (K F)]
            sync.dma_start(
                wsb[:, :], w.ap().rearrange("k c f -> c k f")
            ).then_inc(wsem, 16)
            for g in range(G):
                b = g % BPC
                if g >= 2:
                    # buffer g%2 must be fully consumed by PE (pass g-2)
                    sync.wait_ge(pe_sem, NBANK * NSWEEP * (g - 1))
                xb = xbufs[g % 2]
                for c in range(XDMA):
                    sync.dma_start(
                        xb[:, c * XCOLS : (c + 1) * XCOLS],
                        xT[b, :, c * XCOLS : (c + 1) * XCOLS],
                    ).then_inc(xsems[c], 16)
            # leave all semaphores at 0 so the NEFF can be re-executed
            for sl in range(NOBUF):
                sync.wait_ge(osems[sl], 16 * (TTS // NOBUF))
            for s in [wsem, pe_sem, dve_sem, act_sem] + xsems + osems:
                sync.sem_clear(s)

        @block.tensor
        def _(tensor):
            tensor.wait_ge(wsem, 16)
            xseen = [0] * XDMA
            for g in range(G):
                xb = xbufs[g % 2]
                for s in range(NSWEEP):
                    S = g * NSWEEP + s
                    need = 16 * (g + 1)
                    last_col = min(L, (NBANK * s + NBANK) * CHUNK + K - 1)
                    for c in range(XDMA):
                        if c * XCOLS < last_col and xseen[c] < need:
                            tensor.wait_ge(xsems[c], need)
                            xseen[c] = need
                    for k in range(K):
                        for j in range(NBANK):
                            cj = NBANK * s + j
                            n = _chunk_n(cj)
                            if k == 0 and S >= 1:
                                drain_wait(tensor, S - 1, j)
                            ins = nc.tensor.matmul(
                                psum[:, j * CHUNK : j * CHUNK + n],
                                wsb[:, k * F : (k + 1) * F],
                                xb[:, cj * CHUNK + k : cj * CHUNK + k + n],
                                start=(k == 0),
                                stop=(k == K - 1),
                                skip_group_check=True,
                            )
                            if k == K - 1:
                                ins.then_inc(pe_sem, 1)

        def drain_body(engine, copy_fn, banks, sem):
            for S in range(TTS):
                s = S % NSWEEP
                slot = S % NOBUF
                if S >= NOBUF:
                    engine.wait_ge(osems[slot], 16 * (S // NOBUF))
                for j in banks:
                    n = _chunk_n(NBANK * s + j)
                    engine.wait_ge(pe_sem, NBANK * S + j + 1)
                    copy_fn(
                        obuf[:, slot * SWCOLS + j * CHUNK :
                             slot * SWCOLS + j * CHUNK + n],
                        psum[:, j * CHUNK : j * CHUNK + n],
                    ).then_inc(sem, 1)

        @block.vector
        def _(vector):
            drain_body(vector, nc.vector.tensor_copy, DVE_BANKS, dve_sem)

        @block.scalar
        def _(scalar):
            for S in range(TTS):
                s = S % NSWEEP
                slot = S % NOBUF
                if S >= NOBUF:
                    scalar.wait_ge(osems[slot], 16 * (S // NOBUF))
                for j in ACT_BANKS:
                    n = _chunk_n(NBANK * s + j)
                    scalar.wait_ge(pe_sem, NBANK * S + j + 1)
                    nc.scalar.copy(
                        obuf[:, slot * SWCOLS + j * CHUNK :
                             slot * SWCOLS + j * CHUNK + n],
                        psum[:, j * CHUNK : j * CHUNK + n],
                    ).then_inc(act_sem, 1)
                _store(scalar, S)

    return nc


def make_in_maps(x, kernel):
    """Host-side prep: per-batch transpose to [C, L] and bf16 cast."""
    x = np.asarray(x, dtype=np.float32)
    w = np.ascontiguousarray(np.asarray(kernel, dtype=np.float32)).astype(NPBF16)
    xT = np.ascontiguousarray(
        np.transpose(x[..., 0], (0, 2, 1))
    ).astype(NPBF16)
    return [
        {"xT": xT[i * BPC : (i + 1) * BPC], "w": w} for i in range(NCORES)
    ]


def kernel(x, kernel):
    global _nc
    in_maps = make_in_maps(x, kernel)
    if _nc is None:
        _nc = _build()
    res = run_bass_kernel_spmd(_nc, in_maps, core_ids=list(range(NCORES)))
    outT = np.concatenate(
        [r["outT"].astype(np.float32) for r in res.results], axis=0
    )  # [B, F, LOUT]
    out = np.transpose(outT, (0, 2, 1))[..., None]
    return np.ascontiguousarray(out).astype(np.float32)


# revision 17
# speedup vs baseline: 1.0655x; 1.0655x over previous
"""PairwiseConv1D (valid 1D conv, NWC x WIO -> NWC) on 8 TRN2 NeuronCores.

Strategy (v4: bf16 inputs, full-PSUM k-inner sweeps, 3-engine drains):
  - Data-parallel over batch: B=32 -> 4 batches per core, kernel replicated.
  - Host casts x and w to bf16 and feeds x transposed per batch ([C, L]) so
    the contraction dim C sits on SBUF partitions. Input HBM traffic halves
    (2.3e-3 rel err); output stays f32 (dtype-converting PSUM drains
    measured slower and stall the PE).
  - out.T[f, i] = sum_k w[k].T @ xT[:, i+k] as 7 accumulating matmuls per
    512-wide output chunk (PSUM bank = 512 fp32).
  - k-inner sweeps over all 8 PSUM banks (for k: for bank j: matmul): each
    stationary w[k] serves 8*512 moving columns per load, halving the
    serial weight-reload cost vs a 4-chunk interleave (loads measured
    non-overlappable: explicit ldweights prefetch gained nothing).
  - The 8-bank end-of-sweep drain burst is met by splitting the f32
    PSUM->SBUF drains across three engines: DVE {0,3,6}, ACT {1,4,7},
    GPSIMD {2,5}; each bank's drain must land within ~1.5us of the bank
    completing, before the next sweep reuses the bank.
  - All HBM DMA (x loads, weight load, output stores) issued from SP.
"""

import numpy as np
import ml_dtypes

import concourse.bass as bass
import concourse.mybir as mybir
from concourse.bass_utils import run_bass_kernel_spmd

B, L, C, K, F = 32, 8192, 128, 7, 128
NCORES = 8
BPC = B // NCORES  # batches per core
LOUT = L - K + 1  # 8186
CHUNK = 512  # PSUM bank width in fp32
NBANK = 8  # PSUM banks = chunks per sweep
NCHUNK = (LOUT + CHUNK - 1) // CHUNK  # 16 chunks per batch, last = 506
NSWEEP = NCHUNK // NBANK  # 2 sweeps per batch
SWCOLS = NBANK * CHUNK  # 4096 output cols per sweep / store group
XDMA = 4  # x-load DMAs per batch
XCOLS = L // XDMA  # 2048 cols per x DMA
NOBUF = 2  # output group slots (1 group = 1 sweep)

# drain engine assignment: bank j -> (engine, index within sweep)
# (GPSIMD cannot access PSUM, so only DVE and ACT can drain)
DVE_BANKS = (0, 2, 4, 6)
ACT_BANKS = (1, 3, 5, 7)

BF16 = mybir.dt.bfloat16
NPBF16 = ml_dtypes.bfloat16

_nc = None


def _chunk_n(cj):
    return CHUNK if cj < NCHUNK - 1 else LOUT - (NCHUNK - 1) * CHUNK


def _build(reps=1, detect_races=True):
    f32 = mybir.dt.float32
    nc = bass.Bass(detect_race_conditions=detect_races)
    xT = nc.dram_tensor("xT", [BPC, C, L], BF16, kind="ExternalInput")
    w = nc.dram_tensor("w", [K, C, F], BF16, kind="ExternalInput")
    outT = nc.dram_tensor("outT", [BPC, F, LOUT], f32, kind="ExternalOutput")

    G = reps * BPC  # total batch passes
    TTS = G * NSWEEP  # total sweeps

    from contextlib import ExitStack

    with ExitStack() as ctx:
        wsb = ctx.enter_context(nc.sbuf_tensor([C, K * F], BF16))
        xbuf0 = ctx.enter_context(nc.sbuf_tensor([C, L], BF16))
        xbuf1 = ctx.enter_context(nc.sbuf_tensor([C, L], BF16))
        obuf = ctx.enter_context(nc.sbuf_tensor([F, NOBUF * SWCOLS], f32))
        psum = ctx.enter_context(nc.psum_tensor([F, NBANK * CHUNK], f32))
        wsem = ctx.enter_context(nc.semaphore())
        # per-x-DMA-slot sems: counting one sem per slot keeps waits safe
        # against out-of-order completion across DMA queues
        xsems = [
            ctx.enter_context(nc.semaphore(name=f"xsem{c}")) for c in range(XDMA)
        ]
        pe_sem = ctx.enter_context(nc.semaphore())
        dve_sem = ctx.enter_context(nc.semaphore())
        act_sem = ctx.enter_context(nc.semaphore())
        osems = [
            ctx.enter_context(nc.semaphore(name=f"osem{s}")) for s in range(NOBUF)
        ]
        block = ctx.enter_context(nc.Block())

        xbufs = [xbuf0, xbuf1]

        def drain_wait(engine, S, j):
            # bank j of sweep S drained?
            for sem, banks in (
                (dve_sem, DVE_BANKS),
                (act_sem, ACT_BANKS),
            ):
                if j in banks:
                    engine.wait_ge(sem, len(banks) * S + banks.index(j) + 1)
                    return

        def _store(sync, S):
            b = (S // NSWEEP) % BPC
            s = S % NSWEEP
            cols0 = s * SWCOLS
            ncols = min(SWCOLS, LOUT - cols0)
            slot = S % NOBUF
            sync.wait_ge(dve_sem, len(DVE_BANKS) * (S + 1))
            sync.wait_ge(act_sem, len(ACT_BANKS) * (S + 1))
            sync.dma_start(
                outT[b, :, cols0 : cols0 + ncols],
                obuf[:, slot * SWCOLS : slot * SWCOLS + ncols],
            ).then_inc(osems[slot], 16)

        @block.sync
        def _(sync):
            # weights: [K, C, F] -> SBUF [C, (K F)]
            sync.dma_start(
                wsb[:, :], w.ap().rearrange("k c f -> c k f")
            ).then_inc(wsem, 16)
            for g in range(G):
                b = g % BPC
                if g >= 2:
                    # buffer g%2 must be fully consumed by PE (pass g-2)
                    sync.wait_ge(pe_sem, NBANK * NSWEEP * (g - 1))
                xb = xbufs[g % 2]
                for c in range(XDMA):
                    sync.dma_start(
                        xb[:, c * XCOLS : (c + 1) * XCOLS],
                        xT[b, :, c * XCOLS : (c + 1) * XCOLS],
                    ).then_inc(xsems[c], 16)
                # output stores for the previous pass
                if g >= 1:
                    for s in range(NSWEEP):
                        _store(sync, (g - 1) * NSWEEP + s)
            for s in range(NSWEEP):
                _store(sync, (G - 1) * NSWEEP + s)
            # leave all semaphores at 0 so the NEFF can be re-executed
            for sl in range(NOBUF):
                sync.wait_ge(osems[sl], 16 * (TTS // NOBUF))
            for s in [wsem, pe_sem, dve_sem, act_sem] + xsems + osems:
                sync.sem_clear(s)

        @block.tensor
        def _(tensor):
            tensor.wait_ge(wsem, 16)
            xseen = [0] * XDMA
            for g in range(G):
                xb = xbufs[g % 2]
                for s in range(NSWEEP):
                    S = g * NSWEEP + s
                    need = 16 * (g + 1)
                    last_col = min(L, (NBANK * s + NBANK) * CHUNK + K - 1)
                    for c in range(XDMA):
                        if c * XCOLS < last_col and xseen[c] < need:
                            tensor.wait_ge(xsems[c], need)
                            xseen[c] = need
                    for k in range(K):
                        for j in range(NBANK):
                            cj = NBANK * s + j
                            n = _chunk_n(cj)
                            if k == 0 and S >= 1:
                                drain_wait(tensor, S - 1, j)
                            ins = nc.tensor.matmul(
                                psum[:, j * CHUNK : j * CHUNK + n],
                                wsb[:, k * F : (k + 1) * F],
                                xb[:, cj * CHUNK + k : cj * CHUNK + k + n],
                                start=(k == 0),
                                stop=(k == K - 1),
                                skip_group_check=True,
                            )
                            if k == K - 1:
                                ins.then_inc(pe_sem, 1)

        def drain_body(engine, copy_fn, banks, sem):
            for S in range(TTS):
                s = S % NSWEEP
                slot = S % NOBUF
                if S >= NOBUF:
                    engine.wait_ge(osems[slot], 16 * (S // NOBUF))
                for j in banks:
                    n = _chunk_n(NBANK * s + j)
                    engine.wait_ge(pe_sem, NBANK * S + j + 1)
                    copy_fn(
                        obuf[:, slot * SWCOLS + j * CHUNK :
                             slot * SWCOLS + j * CHUNK + n],
                        psum[:, j * CHUNK : j * CHUNK + n],
                    ).then_inc(sem, 1)

        @block.vector
        def _(vector):
            drain_body(vector, nc.vector.tensor_copy, DVE_BANKS, dve_sem)

        @block.scalar
        def _(scalar):
            drain_body(scalar, nc.scalar.copy, ACT_BANKS, act_sem)

    return nc


def make_in_maps(x, kernel):
    """Host-side prep: per-batch transpose to [C, L] and bf16 cast."""
    x = np.asarray(x, dtype=np.float32)
    w = np.ascontiguousarray(np.asarray(kernel, dtype=np.float32)).astype(NPBF16)
    xT = np.ascontiguousarray(
        np.transpose(x[..., 0], (0, 2, 1))
    ).astype(NPBF16)
    return [
        {"xT": xT[i * BPC : (i + 1) * BPC], "w": w} for i in range(NCORES)
    ]


def kernel(x, kernel):
    global _nc
    in_maps = make_in_maps(x, kernel)
    if _nc is None:
        _nc = _build()
    res = run_bass_kernel_spmd(_nc, in_maps, core_ids=list(range(NCORES)))
    outT = np.concatenate(
        [r["outT"].astype(np.float32) for r in res.results], axis=0
    )  # [B, F, LOUT]
    out = np.transpose(outT, (0, 2, 1))[..., None]
    return np.ascontiguousarray(out).astype(np.float32)


# revision 18
# speedup vs baseline: 1.0680x; 1.0023x over previous
"""PairwiseConv1D (valid 1D conv, NWC x WIO -> NWC) on 8 TRN2 NeuronCores.

Strategy (v4: bf16 inputs, full-PSUM k-inner sweeps, split drains):
  - Data-parallel over batch: B=32 -> 4 batches per core, kernel replicated.
  - Host casts x and w to bf16 and feeds x transposed per batch ([C, L]) so
    the contraction dim C sits on SBUF partitions. Input HBM traffic halves
    (2.3e-3 rel err); output stays f32 (dtype-converting PSUM drains
    measured slower and stall the PE).
  - out.T[f, i] = sum_k w[k].T @ xT[:, i+k] as 7 accumulating matmuls per
    512-wide output chunk (PSUM bank = 512 fp32).
  - k-inner sweeps over all 8 PSUM banks (for k: for bank j: matmul): each
    stationary w[k] serves 8*512 moving columns per load, halving the
    serial weight-reload cost vs a 4-chunk interleave (loads measured
    non-overlappable: explicit ldweights prefetch gained nothing).
  - The 8-bank end-of-sweep drain burst is met by splitting the f32
    PSUM->SBUF drains across two engines: DVE takes even banks, ACT odd
    banks; each bank's drain must land within ~1.5us of the bank
    completing, before the next sweep's start matmul reuses the bank.
    (GPSIMD cannot access PSUM; ACT- or GPSIMD-issued store DMAs and
    dtype-converting drains all measured slower.)
  - All HBM DMA (x loads, weight load, output stores) issued from SP.
"""

import numpy as np
import ml_dtypes

import concourse.bass as bass
import concourse.mybir as mybir
from concourse.bass_utils import run_bass_kernel_spmd

B, L, C, K, F = 32, 8192, 128, 7, 128
NCORES = 8
BPC = B // NCORES  # batches per core
LOUT = L - K + 1  # 8186
CHUNK = 512  # PSUM bank width in fp32
NBANK = 8  # PSUM banks = chunks per sweep
NCHUNK = (LOUT + CHUNK - 1) // CHUNK  # 16 chunks per batch, last = 506
NSWEEP = NCHUNK // NBANK  # 2 sweeps per batch
SWCOLS = NBANK * CHUNK  # 4096 output cols per sweep / store group
XDMA = 4  # x-load DMAs per batch
XCOLS = L // XDMA  # 2048 cols per x DMA
NOBUF = 2  # output group slots (1 group = 1 sweep)

# drain engine assignment: bank j -> (engine, index within sweep)
# (GPSIMD cannot access PSUM, so only DVE and ACT can drain)
DVE_BANKS = (0, 2, 4, 6)
ACT_BANKS = (1, 3, 5, 7)

BF16 = mybir.dt.bfloat16
NPBF16 = ml_dtypes.bfloat16

_nc = None


def _chunk_n(cj):
    return CHUNK if cj < NCHUNK - 1 else LOUT - (NCHUNK - 1) * CHUNK


def _build(reps=1, detect_races=True):
    f32 = mybir.dt.float32
    nc = bass.Bass(detect_race_conditions=detect_races)
    xT = nc.dram_tensor("xT", [BPC, C, L], BF16, kind="ExternalInput")
    w = nc.dram_tensor("w", [K, C, F], BF16, kind="ExternalInput")
    outT = nc.dram_tensor("outT", [BPC, F, LOUT], f32, kind="ExternalOutput")

    G = reps * BPC  # total batch passes
    TTS = G * NSWEEP  # total sweeps

    from contextlib import ExitStack

    with ExitStack() as ctx:
        wsb = ctx.enter_context(nc.sbuf_tensor([C, K * F], BF16))
        xbuf0 = ctx.enter_context(nc.sbuf_tensor([C, L], BF16))
        xbuf1 = ctx.enter_context(nc.sbuf_tensor([C, L], BF16))
        obuf = ctx.enter_context(nc.sbuf_tensor([F, NOBUF * SWCOLS], f32))
        psum = ctx.enter_context(nc.psum_tensor([F, NBANK * CHUNK], f32))
        wsem = ctx.enter_context(nc.semaphore())
        # per-x-DMA-slot sems: counting one sem per slot keeps waits safe
        # against out-of-order completion across DMA queues
        xsems = [
            ctx.enter_context(nc.semaphore(name=f"xsem{c}")) for c in range(XDMA)
        ]
        pe_sem = ctx.enter_context(nc.semaphore())
        dve_sem = ctx.enter_context(nc.semaphore())
        act_sem = ctx.enter_context(nc.semaphore())
        osems = [
            ctx.enter_context(nc.semaphore(name=f"osem{s}")) for s in range(NOBUF)
        ]
        block = ctx.enter_context(nc.Block())

        xbufs = [xbuf0, xbuf1]

        def drain_wait(engine, S, j):
            # bank j of sweep S drained?
            for sem, banks in (
                (dve_sem, DVE_BANKS),
                (act_sem, ACT_BANKS),
            ):
                if j in banks:
                    engine.wait_ge(sem, len(banks) * S + banks.index(j) + 1)
                    return

        def _store(sync, S):
            b = (S // NSWEEP) % BPC
            s = S % NSWEEP
            cols0 = s * SWCOLS
            ncols = min(SWCOLS, LOUT - cols0)
            slot = S % NOBUF
            sync.wait_ge(dve_sem, len(DVE_BANKS) * (S + 1))
            sync.wait_ge(act_sem, len(ACT_BANKS) * (S + 1))
            sync.dma_start(
                outT[b, :, cols0 : cols0 + ncols],
                obuf[:, slot * SWCOLS : slot * SWCOLS + ncols],
            ).then_inc(osems[slot], 16)

        @block.sync
        def _(sync):
            # weights: [K, C, F] -> SBUF [C, (K F)]
            sync.dma_start(
                wsb[:, :], w.ap().rearrange("k c f -> c k f")
            ).then_inc(wsem, 16)
            for g in range(G):
                b = g % BPC
                if g >= 2:
                    # buffer g%2 must be fully consumed by PE (pass g-2)
                    sync.wait_ge(pe_sem, NBANK * NSWEEP * (g - 1))
                xb = xbufs[g % 2]
                for c in range(XDMA):
                    sync.dma_start(
                        xb[:, c * XCOLS : (c + 1) * XCOLS],
                        xT[b, :, c * XCOLS : (c + 1) * XCOLS],
                    ).then_inc(xsems[c], 16)
                # output stores for the previous pass
                if g >= 1:
                    for s in range(NSWEEP):
                        _store(sync, (g - 1) * NSWEEP + s)
            for s in range(NSWEEP):
                _store(sync, (G - 1) * NSWEEP + s)
            # leave all semaphores at 0 so the NEFF can be re-executed
            for sl in range(NOBUF):
                sync.wait_ge(osems[sl], 16 * (TTS // NOBUF))
            for s in [wsem, pe_sem, dve_sem, act_sem] + xsems + osems:
                sync.sem_clear(s)

        @block.tensor
        def _(tensor):
            tensor.wait_ge(wsem, 16)
            xseen = [0] * XDMA
            for g in range(G):
                xb = xbufs[g % 2]
                for s in range(NSWEEP):
                    S = g * NSWEEP + s
                    need = 16 * (g + 1)
                    last_col = min(L, (NBANK * s + NBANK) * CHUNK + K - 1)
                    for c in range(XDMA):
                        if c * XCOLS < last_col and xseen[c] < need:
                            tensor.wait_ge(xsems[c], need)
                            xseen[c] = need
                    for k in range(K):
                        for j in range(NBANK):
                            cj = NBANK * s + j
                            n = _chunk_n(cj)
                            if k == 0 and S >= 1:
                                drain_wait(tensor, S - 1, j)
                            ins = nc.tensor.matmul(
                                psum[:, j * CHUNK : j * CHUNK + n],
                                wsb[:, k * F : (k + 1) * F],
                                xb[:, cj * CHUNK + k : cj * CHUNK + k + n],
                                start=(k == 0),
                                stop=(k == K - 1),
                                skip_group_check=True,
                            )
                            if k == K - 1:
                                ins.then_inc(pe_sem, 1)

        def drain_body(engine, copy_fn, banks, sem):
            for S in range(TTS):
                s = S % NSWEEP
                slot = S % NOBUF
                if S >= NOBUF:
                    engine.wait_ge(osems[slot], 16 * (S // NOBUF))
                for j in banks:
                    n = _chunk_n(NBANK * s + j)
                    engine.wait_ge(pe_sem, NBANK * S + j + 1)
                    copy_fn(
                        obuf[:, slot * SWCOLS + j * CHUNK :
                             slot * SWCOLS + j * CHUNK + n],
                        psum[:, j * CHUNK : j * CHUNK + n],
                    ).then_inc(sem, 1)

        @block.vector
        def _(vector):
            drain_body(vector, nc.vector.tensor_copy, DVE_BANKS, dve_sem)

        @block.scalar
        def _(scalar):
            drain_body(scalar, nc.scalar.copy, ACT_BANKS, act_sem)

    return nc


def make_in_maps(x, kernel):
    """Host-side prep: per-batch transpose to [C, L] and bf16 cast."""
    x = np.asarray(x, dtype=np.float32)
    w = np.ascontiguousarray(np.asarray(kernel, dtype=np.float32)).astype(NPBF16)
    xT = np.ascontiguousarray(
        np.transpose(x[..., 0], (0, 2, 1))
    ).astype(NPBF16)
    return [
        {"xT": xT[i * BPC : (i + 1) * BPC], "w": w} for i in range(NCORES)
    ]


def kernel(x, kernel):
    global _nc
    in_maps = make_in_maps(x, kernel)
    if _nc is None:
        _nc = _build()
    res = run_bass_kernel_spmd(_nc, in_maps, core_ids=list(range(NCORES)))
    outT = np.concatenate(
        [r["outT"].astype(np.float32) for r in res.results], axis=0
    )  # [B, F, LOUT]
    out = np.transpose(outT, (0, 2, 1))[..., None]
    return np.ascontiguousarray(out).astype(np.float32)
